# revision 1
# baseline (speedup 1.0000x reference)
"""Trainium2 Bass kernel for nn_KeypointsLoss.

Math (per batch b):
    x[p,k] = trunc(kp[b,p,k,0] * (W-1)); y likewise from kp[...,1]
    g_row[p,k,h] = exp(-(h-x)^2/(2s^2)) * (vis>0);  g_col[p,k,w] = exp(-(w-y)^2/(2s^2))
    target[k] = sum_p outer(g_row, g_col)            # [H,W]
    per_sample = sum_k |pred[b,k] - target[k]|^2
    loss = sum_b per_sample / (sum(vis[b]) + 1e-6) / B

Strategy (8 cores, data-parallel over B=32 -> 4 batches/core):
  - PE builds (target - pred) directly in PSUM: block-diag g_col matmuls splat
    the 4-k-group targets, then a (-I96) matmul accumulates -pred.
  - ScalarE (activation Square + fused row-sum) / VectorE (tensor_tensor_reduce)
    square-reduce PSUM into per-batch accumulators.
  - Tiny matmul with a (1/32)-vector reduces partitions; visibility normalizer
    computed on-device; host just sums the 8x4 partials.
"""

import sys
import numpy as np

sys.path.insert(0, "/opt/trn_rl_repo")

B, P, K, H, W = 32, 8, 17, 192, 192
SIGMA = 3.0
INV2S2 = 1.0 / (2.0 * SIGMA**2)
NCORES = 8
NB = B // NCORES          # batches per core
HL = 96                   # h split: [0:96) lo, [96:192) hi
KW = K * W                # 3264 free cols for pred tiles
NG = 4                    # full k-groups of 4 (k0..15); k=16 handled separately

_CACHE = {}


def _build():
    import concourse.bass as bass
    import concourse.bacc as bacc
    import concourse.tile as tile
    from concourse import mybir

    f32 = mybir.dt.float32
    bf16 = mybir.dt.bfloat16
    i32 = mybir.dt.int32
    Alu = mybir.AluOpType
    Act = mybir.ActivationFunctionType

    nc = bacc.Bacc("TRN2", target_bir_lowering=False, debug=False,
                   num_devices=NCORES)

    pred_d = nc.dram_tensor("pred", [NB, K, H, W], f32, kind="ExternalInput").ap()
    kp_d = nc.dram_tensor("kp", [NB, K, P, 2], f32, kind="ExternalInput").ap()
    vis_d = nc.dram_tensor("vis", [NB, K, P], i32, kind="ExternalInput").ap()
    negi_d = nc.dram_tensor("negi", [96, 96], bf16, kind="ExternalInput").ap()
    iota_d = nc.dram_tensor("iota", [128, W], f32, kind="ExternalInput").ap()
    out_d = nc.dram_tensor("out", [NB, 1], f32, kind="ExternalOutput").ap()

    with tile.TileContext(nc) as tc:
        import contextlib
        with contextlib.ExitStack() as ctx:
            consts = ctx.enter_context(tc.tile_pool(name="consts", bufs=1))
            gpool = ctx.enter_context(tc.tile_pool(name="gpool", bufs=1))
            colp = ctx.enter_context(tc.tile_pool(name="cols", bufs=1))
            genp = ctx.enter_context(tc.tile_pool(name="gen", bufs=2))
            predp = ctx.enter_context(tc.tile_pool(name="pred", bufs=2))
            scrp = ctx.enter_context(tc.tile_pool(name="scr", bufs=2))
            psump = ctx.enter_context(tc.tile_pool(name="psum", bufs=2, space="PSUM"))

            iota_t = consts.tile([128, W], f32, tag="iota")
            negi_t = consts.tile([96, 96], bf16, tag="negi")
            ones_t = consts.tile([96, 1], f32, tag="ones")
            accall = consts.tile([96, NB], f32, tag="accall")

            # pred, cast to bf16 during DMA (SWDGE), per batch so each b's
            # completion sem fires as soon as its own slice lands
            plo_t, phi_t = [], []
            for b in range(NB):
                psrc = pred_d[b].rearrange("k h w -> h k w")
                plo = predp.tile([HL, KW], bf16, tag=f"plo{b}", name=f"plo{b}")
                phi = predp.tile([HL, KW], bf16, tag=f"phi{b}", name=f"phi{b}")
                nc.gpsimd.dma_start(
                    out=plo[:].rearrange("p (k w) -> p k w", w=W),
                    in_=psrc[0:HL])
                nc.gpsimd.dma_start(
                    out=phi[:].rearrange("p (k w) -> p k w", w=W),
                    in_=psrc[HL:H])
                plo_t.append(plo)
                phi_t.append(phi)

            # --- batched column inputs: x,y per b (cols 2b,2b+1), k16 xy
            # (cols 8,9); vis per b (cols 0..3), k16 vis (col 4)
            kpd = colp.tile([128, 2 * NB], f32, tag="kpd", name="kpd")
            visd = colp.tile([128, NB], i32, tag="visd", name="visd")
            kpt1 = colp.tile([128, 2], f32, tag="kpt1", name="kpt1")
            vist1 = colp.tile([128, 1], i32, tag="vist1", name="vist1")
            # dense kp: [(k p), (b t)] in ONE transfer; dense vis likewise
            nc.sync.dma_start(
                out=kpd[:].rearrange("p (b t) -> p b t", t=2),
                in_=kp_d.rearrange("b k p t -> (k p) b t")[0:128])
            nc.sync.dma_start(out=iota_t[:], in_=iota_d[:])
            nc.sync.dma_start(
                out=visd[:],
                in_=vis_d.rearrange("b k p -> (k p) b")[0:128])
            nc.sync.dma_start(out=negi_t[:], in_=negi_d[:])
            nc.vector.memset(kpt1[:], 0.0)
            nc.vector.memset(vist1[:], 0)
            nc.vector.memset(ones_t[:], 1.0 / B)

            # per-group block-diag g_col staircase tiles (rows outside a
            # group band are never read: splats contract over c=32 only)
            bd_g = [consts.tile([128, NB * 4 * W], bf16, tag=f"bd_g{g}",
                                name=f"bd_g{g}") for g in range(NG)]
            for g in range(NG):
                nc.vector.memset(bd_g[g][:].bitcast(f32), 0.0)
            gcol_all = consts.tile([128, NB * W], bf16, tag="gcol_all",
                                   name="gcol_all")

            # batched -trunc(kp*191), rounding-agnostic:
            # xf = round_any(t); trunc = xf - (xf > t); neg = -trunc
            def trunc_chain(kp_src, n, nm):
                tall = colp.tile([128, n], f32, tag=f"t_{nm}", name=f"t_{nm}")
                nall = colp.tile([128, n], f32, tag=f"n_{nm}", name=f"n_{nm}")
                xi = colp.tile([128, n], i32, tag=f"xi_{nm}", name=f"xi_{nm}")
                xf = colp.tile([128, n], f32, tag=f"xf_{nm}", name=f"xf_{nm}")
                nc.scalar.mul(tall[:], kp_src, float(W - 1))
                nc.vector.tensor_copy(xi[:], tall[:])
                nc.vector.tensor_copy(xf[:], xi[:])
                nc.vector.tensor_tensor(nall[:], xf[:], tall[:], Alu.is_gt)
                nc.vector.tensor_tensor(nall[:], nall[:], xf[:], Alu.subtract)
                return nall
            negd = trunc_chain(kpd[:], 2 * NB, "d")
            visfd = colp.tile([128, NB], f32, tag="visfd", name="visfd")
            nc.vector.tensor_copy(visfd[:], visd[:])

            def gen_g(dst, negcol, viscol=None, on_scalar=False):
                dx2 = genp.tile([128, W], f32, tag="gen_dx2", name="gen_dx2")
                if on_scalar:
                    nc.scalar.activation(dx2[:], iota_t[:], Act.Square,
                                         bias=negcol)
                else:
                    dx = genp.tile([128, W], f32, tag="gen_dx", name="gen_dx")
                    nc.vector.tensor_scalar_add(dx[:], iota_t[:], negcol)
                    nc.vector.tensor_tensor(dx2[:], dx[:], dx[:], Alu.mult)
                nc.scalar.activation(dst[:], dx2[:], Act.Exp, scale=-INV2S2)
                if viscol is not None:
                    nc.vector.tensor_scalar_mul(dst[:], dst[:], viscol)

            # dense tiles: partition = 8*k + p for k in [0,16)
            grow0 = [gpool.tile([128, W], bf16, tag=f"grow0_{b}", name=f"grow0_{b}") for b in range(NB)]
            gcol0 = [gcol_all[:, b * W:(b + 1) * W] for b in range(NB)]
            # gcol for all b first (scalar path): it gates the staircase DMAs
            for b in range(NB):
                gen_g(gcol0[b], negd[:, 2 * b + 1:2 * b + 2], on_scalar=True)
            # staircase: one DMA per k covers all NB batches, split over the
            # two HWDGE issue queues (sync + scalar)
            gcv = gcol_all[:].rearrange("p (b c) -> p b c", c=W)
            for k in range(16):
                bdv = bd_g[k // 4][:].rearrange("p (b c) -> p b c", c=4 * W)
                eng = nc.sync if k % 2 == 0 else nc.scalar
                eng.dma_start(
                    out=bdv[8 * k:8 * k + P, :, (k % 4) * W:(k % 4 + 1) * W],
                    in_=gcv[8 * k:8 * k + P, :, :])
            # grow on the vector path
            for b in range(NB):
                gen_g(grow0[b], negd[:, 2 * b:2 * b + 1],
                      visfd[:, b:b + 1])

            # k=16 inputs (gate only the leftover groups, emitted late)
            for b in range(NB):
                nc.sync.dma_start(out=kpt1[32 * b:32 * b + P, 0:2],
                                  in_=kp_d[b, 16, :, 0:2])
                nc.sync.dma_start(out=vist1[32 * b:32 * b + P, 0:1],
                                  in_=vis_d[b, 16, :][:, None])
            negt1 = trunc_chain(kpt1[:], 2, "t1")
            visft1 = colp.tile([128, 1], f32, tag="visft1", name="visft1")
            nc.vector.tensor_copy(visft1[:], vist1[:])
            # k=16 tiles: partition = 32*b + p
            grow1 = gpool.tile([128, W], bf16, tag="grow1")
            gcol1 = gpool.tile([128, W], bf16, tag="gcol1")
            gen_g(gcol1, negt1[:, 1:2])
            gen_g(grow1, negt1[:, 0:1], visft1[:, 0:1])

            # ---------------- main loop ----------------
            accs_t = [gpool.tile([96, NG + 1], f32, tag=f"accs{b}",
                                 name=f"accs{b}") for b in range(NB)]
            for b in range(NB):
                plo = plo_t[b]
                phi = phi_t[b]
                accs = accs_t[b]
                for g in range(NG):
                    vector_group = g == 1 or (g == 2 and b == 3)
                    ps = psump.tile([96, 2048], f32, tag="ps", name="ps")
                    lo = grow0[b][:, 0:HL]
                    hi = grow0[b][:, HL:H]
                    bdt = bd_g[g][:, b * 4 * W:(b + 1) * 4 * W]
                    c0 = g * 4 * W  # start col in (k,w) space for this group
                    # splat targets (pairs of k share one bank); full c=128 so
                    # the HAM clock-gate sees full-array matmuls (row-banded
                    # tile_position MMs do not warm the PE clock)
                    nc.tensor.matmul(ps[:, 0:384], lo, bdt[:, 0:384],
                                     start=True, stop=vector_group)
                    nc.tensor.matmul(ps[:, 512:896], lo, bdt[:, 384:768],
                                     start=True, stop=vector_group)
                    nc.tensor.matmul(ps[:, 1024:1408], hi, bdt[:, 0:384],
                                     start=True, stop=vector_group)
                    nc.tensor.matmul(ps[:, 1536:1920], hi, bdt[:, 384:768],
                                     start=True, stop=vector_group)
                    if vector_group:
                        # DVE path: psum holds target only; subtract pred on DVE
                        # (one PSUM input allowed), square+reduce from SBUF.
                        diff = scrp.tile([96, 4 * 384], f32, tag="diff", name="diff")
                        psv = ps[:].rearrange("p (a c) -> p a c", c=512)
                        pslo = psv[:, 0:2, 0:384]
                        pshi = psv[:, 2:4, 0:384]
                        dlo = diff[:, 0:768].rearrange("p (a c) -> p a c", c=384)
                        dhi = diff[:, 768:1536].rearrange("p (a c) -> p a c", c=384)
                        plov = plo[:, c0:c0 + 768].rearrange(
                            "p (a c) -> p a c", c=384)
                        phiv = phi[:, c0:c0 + 768].rearrange(
                            "p (a c) -> p a c", c=384)
                        nc.vector.tensor_tensor(dlo, pslo, plov, Alu.subtract)
                        nc.vector.tensor_tensor(dhi, pshi, phiv, Alu.subtract)
                        scr = scrp.tile([96, 4 * 384], f32, tag="scr", name="scr")
                        nc.vector.affine_mul_reduce(
                            out=scr[:], accum_out=accs[:, g:g + 1],
                            in0=diff[:], in1=diff[:], scale=1.0, bias=0.0)
                    else:
                        # accumulate -pred on PE, square+reduce on ScalarE
                        nc.tensor.matmul(ps[:, 0:384], negi_t[:],
                                         plo[:, c0:c0 + 384],
                                         start=False, stop=True)
                        nc.tensor.matmul(ps[:, 512:896], negi_t[:],
                                         plo[:, c0 + 384:c0 + 768],
                                         start=False, stop=True)
                        nc.tensor.matmul(ps[:, 1024:1408], negi_t[:],
                                         phi[:, c0:c0 + 384],
                                         start=False, stop=True)
                        nc.tensor.matmul(ps[:, 1536:1920], negi_t[:],
                                         phi[:, c0 + 384:c0 + 768],
                                         start=False, stop=True)
                        view = ps[:].rearrange("p (a c) -> p a c", c=512)[:, :, 0:384]
                        scr = scrp.tile([96, 4 * 384], f32, tag="scr", name="scr")
                        sview = scr[:].rearrange("p (a c) -> p a c", c=384)
                        nc.scalar.activation(sview, view, Act.Square,
                                             accum_out=accs[:, g:g + 1])

            # leftover k = 16 for all batches, then per-batch reduce
            for b in range(NB):
                plo = plo_t[b]
                phi = phi_t[b]
                accs = accs_t[b]
                ps = psump.tile([96, 2048], f32, tag="ps", name="ps")
                l1 = grow1[32 * b:32 * b + P, 0:HL]
                h1 = grow1[32 * b:32 * b + P, HL:H]
                gc1 = gcol1[32 * b:32 * b + P, :]
                nc.tensor.matmul(ps[:, 0:192], l1, gc1, start=True, stop=False,
                                 tile_position=(32 * b, 0))
                nc.tensor.matmul(ps[:, 512:704], h1, gc1, start=True, stop=False,
                                 tile_position=(32 * b, 0))
                nc.tensor.matmul(ps[:, 0:192], negi_t[:], plo[:, 16 * W:17 * W],
                                 start=False, stop=True)
                nc.tensor.matmul(ps[:, 512:704], negi_t[:], phi[:, 16 * W:17 * W],
                                 start=False, stop=True)
                lview = ps[:].rearrange("p (a c) -> p a c", c=512)[:, 0:2, 0:192]
                scr = scrp.tile([96, 4 * 384], f32, tag="scr", name="scr")
                lsview = scr[:, 0:384].rearrange("p (a c) -> p a c", c=192)
                nc.scalar.activation(lsview, lview, Act.Square,
                                     accum_out=accs[:, NG:NG + 1])

                nc.vector.tensor_reduce(accall[:, b:b + 1], accs[:],
                                        axis=mybir.AxisListType.X, op=Alu.add)

            # ---------------- finalize ----------------
            ps2 = psump.tile([96, 2048], f32, tag="ps", name="ps")
            nc.tensor.matmul(ps2[0:NB, 0:1], accall[:, 0:NB], ones_t[:],
                             start=True, stop=True)

            vist = colp.tile([NB, P * K], i32, tag="vist")
            nc.sync.dma_start(out=vist[:], in_=vis_d.rearrange("b k p -> b (k p)"))
            visf = colp.tile([NB, P * K], f32, tag="visf")
            nc.vector.tensor_copy(visf[:], vist[:])
            den = colp.tile([NB, 1], f32, tag="den")
            nc.vector.tensor_reduce(den[:], visf[:], axis=mybir.AxisListType.X,
                                    op=Alu.add)
            nc.vector.tensor_scalar_add(den[:], den[:], 1e-6)
            invd = colp.tile([NB, 1], f32, tag="invd")
            nc.vector.reciprocal(invd[:], den[:])
            outt = colp.tile([NB, 1], f32, tag="outt")
            nc.vector.tensor_tensor(outt[:], ps2[0:NB, 0:1], invd[:], Alu.mult)
            nc.sync.dma_start(out=out_d[:], in_=outt[:])

    nc.compile()
    return nc


def get_nc():
    if "nc" not in _CACHE:
        _CACHE["nc"] = _build()
    return _CACHE["nc"]


def make_in_maps(pred_heatmaps, keypoints, visibilities):
    pred = np.ascontiguousarray(pred_heatmaps, dtype=np.float32)
    # device expects [.., K, P, ..] layout so (k p) merges to a contiguous stride
    kp = np.ascontiguousarray(
        np.asarray(keypoints, dtype=np.float32).transpose(0, 2, 1, 3))
    vis = np.ascontiguousarray(
        np.asarray(visibilities, dtype=np.int32).transpose(0, 2, 1))
    import ml_dtypes
    negi = (-np.eye(96)).astype(ml_dtypes.bfloat16)
    iota = np.broadcast_to(np.arange(W, dtype=np.float32), (128, W)).copy()
    in_maps = []
    for c in range(NCORES):
        sl = slice(c * NB, (c + 1) * NB)
        in_maps.append({
            "pred": pred[sl],
            "kp": kp[sl],
            "vis": vis[sl],
            "negi": negi,
            "iota": iota,
        })
    return in_maps


def kernel(pred_heatmaps, keypoints, visibilities):
    from concourse.bass_utils import run_bass_kernel_spmd

    nc = get_nc()
    in_maps = make_in_maps(pred_heatmaps, keypoints, visibilities)
    res = run_bass_kernel_spmd(nc, in_maps, core_ids=list(range(NCORES)))
    total = np.float64(0.0)
    for c in range(NCORES):
        total += np.asarray(res.results[c]["out"], dtype=np.float64).sum()
    return np.float32(total)

